# revision 24
# baseline (speedup 1.0000x reference)
"""Trainium2 Bass kernel for nn_ABS_MHAtt (masked two-round multi-head attention).

Strategy: pure data-parallel over batch (B=16 -> 2 batches per NeuronCore, 8 cores,
no collectives). Host-side preprocessing (inside kernel()) pre-transposes
activations/weights into the [contraction, free] layouts the TensorEngine wants and
pre-converts everything to bf16, so the device kernel does zero layout conversion.

Per-core device kernel (per batch):
  - qhT/khT projections in transposed form [o, i]; v projected in natural form [j, o]
    directly into an "augmented" layout with a ones column per head (the ones column
    makes the PV/AV matmul also produce the softmax row-sum).
  - Per head: scores computed transposed [j, i] (contraction over d=64), exp on
    ScalarE (head pairs share one activation op), masking by multiplying with
    (1-mask)^T (split across VectorE and GpSimdE), PV/AV computed transposed
    ([65, i], E as the moving operand so the weight loads hide), row-sum reciprocal
    broadcast via GpSimd partition_broadcast, and identity-shift matmuls to repack
    the two heads of a pair back onto 128 partitions.
"""

import os
import sys

import numpy as np


def _ensure_concourse():
    try:
        import concourse.bass  # noqa: F401
        return
    except Exception:
        pass
    for p in ("/opt/trn_rl_repo", "/root/.axon_site/_ro/trn_rl_repo"):
        if os.path.isdir(p) and p not in sys.path:
            sys.path.insert(0, p)
            try:
                import concourse.bass  # noqa: F401
                return
            except Exception:
                sys.path.remove(p)
    raise ImportError("cannot import concourse (bass)")


B, L, HS = 16, 512, 1024
H, D = 16, 64
NCORES = 8
BPC = B // NCORES  # batches per core
SCALE = 1.0 / 8.0  # 1/sqrt(D)
AUGW = 65  # per-head augmented width (D + ones column)

_CACHE = {}


def _build_nc():
    _ensure_concourse()
    import concourse.bass as bass  # noqa: F401
    import concourse.mybir as mybir
    import concourse.tile as tile
    from concourse import bacc
    from contextlib import ExitStack

    bf = mybir.dt.bfloat16
    f32 = mybir.dt.float32
    Exp = mybir.ActivationFunctionType.Exp

    nc = bacc.Bacc()

    qt = nc.declare_dram_parameter("qt", [BPC, HS, L], bf, isOutput=False)
    kt = nc.declare_dram_parameter("kt", [BPC, HS, L], bf, isOutput=False)
    vt = nc.declare_dram_parameter("vt", [BPC, HS, L], bf, isOutput=False)
    imt = nc.declare_dram_parameter("imt", [BPC, HS, L], bf, isOutput=False)
    aug = nc.declare_dram_parameter("aug", [BPC, L, H * AUGW], bf, isOutput=False)
    kp1 = nc.declare_dram_parameter("kp1", [BPC, L, L], bf, isOutput=False)
    kp2 = nc.declare_dram_parameter("kp2", [BPC, L, L], bf, isOutput=False)
    wq = nc.declare_dram_parameter("wq", [HS, HS], bf, isOutput=False)
    wk = nc.declare_dram_parameter("wk", [HS, HS], bf, isOutput=False)
    wv = nc.declare_dram_parameter("wv", [HS, HS], bf, isOutput=False)
    wm = nc.declare_dram_parameter("wm", [HS, HS], bf, isOutput=False)
    idt = nc.declare_dram_parameter("idt", [128, 128], bf, isOutput=False)
    ids = nc.declare_dram_parameter("ids", [64, 128], bf, isOutput=False)
    out = nc.declare_dram_parameter("out", [BPC, L, HS], f32, isOutput=True)

    with ExitStack() as ctx:
        tc = ctx.enter_context(tile.TileContext(nc))
        consts = ctx.enter_context(tc.tile_pool(name="consts", bufs=1))
        inp = ctx.enter_context(tc.tile_pool(name="inp", bufs=1))
        proj = ctx.enter_context(tc.tile_pool(name="proj", bufs=1))
        ework = ctx.enter_context(tc.tile_pool(name="ework", bufs=4))
        small = ctx.enter_context(tc.tile_pool(name="small", bufs=6))
        evac = ctx.enter_context(tc.tile_pool(name="evac", bufs=2))
        psA = ctx.enter_context(tc.tile_pool(name="psA", bufs=2, space="PSUM"))
        psPV = ctx.enter_context(tc.tile_pool(name="psPV", bufs=2, space="PSUM"))
        psQN = ctx.enter_context(tc.tile_pool(name="psQN", bufs=2, space="PSUM"))

        w_sb = {}

        def load_weight(name, wext, halves=1):
            t = consts.tile([128, 8, HS], bf, tag=name)
            src_ = wext.rearrange("(t p) o -> p t o", p=128)
            n = 8 // halves
            for i in range(halves):
                nc.sync.dma_start(
                    out=t[:, i * n : (i + 1) * n, :],
                    in_=src_[:, i * n : (i + 1) * n, :],
                )
            w_sb[name] = t

        load_weight("wq", wq, halves=2)
        ident = consts.tile([128, 128], bf, tag="ident")
        idshift = consts.tile([64, 128], bf, tag="idshift")

        for b in range(BPC):
            # Load order matters at startup: the first projection matmuls need
            # wq + qt, so those go first; remaining weights interleave with the
            # batch-0 input loads (HWDGE drains its FIFO in issue order).
            qt_sb = inp.tile([128, 8, L], bf, tag="qt")
            kt_sb = inp.tile([128, 8, L], bf, tag="kt")
            vt_sb = inp.tile([128, 8, L], bf, tag="vt")
            imt_sb = inp.tile([128, 8, L], bf, tag="imt")

            def load_x(t, ext):
                src = ext[b].rearrange("(t p) i -> p t i", p=128)
                for half in range(2):
                    nc.sync.dma_start(
                        out=t[:, half * 4 : (half + 1) * 4, :],
                        in_=src[:, half * 4 : (half + 1) * 4, :],
                    )

            load_x(qt_sb, qt)
            if b == 0:
                load_weight("wk", wk)
            load_x(kt_sb, kt)
            if b == 0:
                load_weight("wv", wv)
            load_x(vt_sb, vt)
            load_x(imt_sb, imt)
            aug_sb = inp.tile([128, 4, H * AUGW], bf, tag="aug")
            nc.sync.dma_start(
                out=aug_sb, in_=aug[b].rearrange("(t p) x -> p t x", p=128)
            )
            kp1_sb = inp.tile([128, 4, L], bf, tag="kp1")
            kp2_sb = inp.tile([128, 4, L], bf, tag="kp2")
            nc.sync.dma_start(
                out=kp1_sb, in_=kp1[b].rearrange("(t p) i -> p t i", p=128)
            )
            nc.sync.dma_start(
                out=kp2_sb, in_=kp2[b].rearrange("(t p) i -> p t i", p=128)
            )
            if b == 0:
                nc.sync.dma_start(out=ident, in_=idt[:, :])
                nc.sync.dma_start(out=idshift, in_=ids[:, :])
                load_weight("wm", wm)

            # ---- projections qhT = Wq @ q^T, khT = Wk @ k^T  (layout [o, i]) ----
            qh_sb = proj.tile([128, 8, L], bf, tag="qh")
            kh_sb = proj.tile([128, 8, L], bf, tag="kh")
            for wname, xsb, dst, ev in (
                ("wq", qt_sb, qh_sb, "act"),
                ("wk", kt_sb, kh_sb, "dve"),
            ):
                wt = w_sb[wname]
                for ot in range(8):
                    ps = psA.tile([128, 512], f32, tag="psA")
                    for kc in range(8):
                        nc.tensor.matmul(
                            ps,
                            wt[:, kc, ot * 128 : (ot + 1) * 128],
                            xsb[:, kc, :],
                            start=(kc == 0),
                            stop=(kc == 7),
                        )
                    if ev == "act":
                        nc.scalar.copy(out=dst[:, ot, :], in_=ps)
                    else:
                        nc.vector.tensor_copy(out=dst[:, ot, :], in_=ps)

            # ---- v projection (natural [j, o]) into augmented layout + ones ----
            vaug_sb = proj.tile([128, 4, H * AUGW], bf, tag="vaug")
            for jt in range(4):
                nc.vector.memset(
                    vaug_sb[:, jt, :].rearrange("p (h x) -> p h x", x=AUGW)[:, :, 64],
                    1.0,
                )
                for oh in range(2):
                    ps = psA.tile([128, 512], f32, tag="psA")
                    for kc in range(8):
                        nc.tensor.matmul(
                            ps,
                            vt_sb[:, kc, jt * 128 : (jt + 1) * 128],
                            w_sb["wv"][:, kc, oh * 512 : (oh + 1) * 512],
                            start=(kc == 0),
                            stop=(kc == 7),
                        )
                    dst_ap = vaug_sb[
                        :, jt, oh * 8 * AUGW : (oh + 1) * 8 * AUGW
                    ].rearrange("p (h x) -> p h x", x=AUGW)[:, :, 0:64]
                    nc.scalar.copy(
                        out=dst_ap, in_=ps.rearrange("p (h x) -> p h x", x=64)
                    )

            att_sb = proj.tile([128, 8, L], bf, tag="att")

            # ---- attention: head pairs, two pairs staggered through each stage ----
            def score_stage(hp, lhs_sb, rhs_fn, kp_sb, etile):
                """s^T [j,i] for both heads of pair hp + exp + mask into etile."""
                heads = (2 * hp, 2 * hp + 1)
                for jt in range(4):
                    ps = psA.tile([128, 1024], f32, tag="psA")
                    for g, h in enumerate(heads):
                        nc.tensor.matmul(
                            ps[:, g * 512 : (g + 1) * 512],
                            lhs_sb[
                                (h % 2) * 64 : (h % 2) * 64 + 64,
                                h // 2,
                                jt * 128 : (jt + 1) * 128,
                            ],
                            rhs_fn(g, h),
                            start=True,
                            stop=True,
                        )
                    nc.scalar.activation(
                        out=etile[:, jt],
                        in_=ps.rearrange("p (g x) -> p g x", x=512),
                        func=Exp,
                        scale=SCALE,
                    )

            def mask_stage(hp, kp_sb, etile):
                # in-place mask multiply, head 1 on the otherwise-idle GpSimd
                for jt in range(4):
                    for g in range(2):
                        eng = nc.gpsimd if g == 1 else nc.vector
                        eng.tensor_mul(
                            etile[:, jt, g], etile[:, jt, g], kp_sb[:, jt, :]
                        )

            def pv_stage(hp, emtile, rhs_sb):
                """pv natural [i, 4*65] per head -> normalized dl pair [128,4,128].

                PSUM evacuated once per head to SBUF (f32) so the reciprocal and
                the per-it tensor_scalar normalizes run SBUF-sourced (2x mode).
                """
                heads = (2 * hp, 2 * hp + 1)
                dl = small.tile([128, 4, 128], bf, tag="dl")
                for g, h in enumerate(heads):
                    pspv = psPV.tile([128, 4 * AUGW], f32, tag="pv")
                    for it in range(4):
                        for jt in range(4):
                            nc.tensor.matmul(
                                pspv[:, it * AUGW : it * AUGW + AUGW],
                                emtile[:, jt, g, it * 128 : (it + 1) * 128],
                                rhs_sb[:, jt, h * AUGW : (h + 1) * AUGW],
                                start=(jt == 0),
                                stop=(jt == 3),
                            )
                    r1 = small.tile([128, 4], f32, tag="r1")
                    nc.vector.reciprocal(
                        r1, pspv.rearrange("p (i x) -> p i x", x=AUGW)[:, :, 64]
                    )
                    for it in range(4):
                        nc.vector.tensor_scalar_mul(
                            dl[:, it, g * 64 : (g + 1) * 64],
                            pspv[:, it * AUGW : it * AUGW + 64],
                            r1[:, it : it + 1],
                        )
                return dl

            def mod_stage(hp, emtile):
                """round-1 tail: pv + normalize + transpose + add qh -> qn_pair."""
                dl = pv_stage(hp, emtile, aug_sb)
                pst = psQN.tile([128, 512], bf, tag="qn")
                for it in range(4):
                    nc.tensor.transpose(
                        pst[:, it * 128 : (it + 1) * 128], dl[:, it], ident
                    )
                qn_pair = small.tile([128, 512], bf, tag="qnp")
                nc.vector.tensor_add(qn_pair, pst, qh_sb[:, hp, :])
                return qn_pair

            def av_stage(hp, emtile):
                """round-2 tail: av + normalize + transpose -> att_sb[:, hp, :]."""
                dl = pv_stage(hp, emtile, vaug_sb)
                pst = psQN.tile([128, 512], bf, tag="qn")
                for it in range(4):
                    nc.tensor.transpose(
                        pst[:, it * 128 : (it + 1) * 128], dl[:, it], ident
                    )
                nc.scalar.copy(out=att_sb[:, hp, :], in_=pst)

            def s1_stage(hp):
                e1 = ework.tile([128, 4, 2, L], bf, tag="e")
                score_stage(
                    hp,
                    imt_sb,
                    lambda g, h: qh_sb[(h % 2) * 64 : (h % 2) * 64 + 64, h // 2, :],
                    kp1_sb,
                    e1,
                )
                mask_stage(hp, kp1_sb, e1)
                return e1

            def s2_stage(hp, qn_pair):
                e2 = ework.tile([128, 4, 2, L], bf, tag="e")
                score_stage(
                    hp,
                    kh_sb,
                    lambda g, h: qn_pair[(h % 2) * 64 : (h % 2) * 64 + 64, :],
                    kp2_sb,
                    e2,
                )
                mask_stage(hp, kp2_sb, e2)
                return e2

            # two-pair software pipeline: each stage runs for 2 pairs before
            # the next stage, so the engines have independent work to overlap
            # the cross-engine dependency chains.
            for base in range(0, 8, 2):
                p0, p1 = base, base + 1
                e1_0 = s1_stage(p0)
                e1_1 = s1_stage(p1)
                qn_0 = mod_stage(p0, e1_0)
                qn_1 = mod_stage(p1, e1_1)
                e2_0 = s2_stage(p0, qn_0)
                e2_1 = s2_stage(p1, qn_1)
                av_stage(p0, e2_0)
                av_stage(p1, e2_1)

            # ---- output projection: out[i, o] = attT^T @ WmT ----
            for it in range(4):
                for oh in range(2):
                    ps = psA.tile([128, 512], f32, tag="psA")
                    for kc in range(8):
                        nc.tensor.matmul(
                            ps,
                            att_sb[:, kc, it * 128 : (it + 1) * 128],
                            w_sb["wm"][:, kc, oh * 512 : (oh + 1) * 512],
                            start=(kc == 0),
                            stop=(kc == 7),
                        )
                    ob = evac.tile([128, 512], f32, tag="ob")
                    nc.vector.tensor_copy(out=ob, in_=ps)
                    nc.sync.dma_start(
                        out=out[b, it * 128 : (it + 1) * 128, oh * 512 : (oh + 1) * 512],
                        in_=ob,
                    )

    nc.compile()
    return nc


def _get_nc():
    if "nc" not in _CACHE:
        _CACHE["nc"] = _build_nc()
    return _CACHE["nc"]


def _prep_inputs(v, k, q, img_abs, Wv, Wk, Wq, Wm, abs_mask, mask):
    import ml_dtypes

    bf16 = ml_dtypes.bfloat16
    f32 = np.float32

    def t_bf(x):  # [B, L, HS] -> [B, HS, L] bf16
        return np.ascontiguousarray(np.swapaxes(np.asarray(x, f32), 1, 2)).astype(bf16)

    qt = t_bf(q)
    ktr = t_bf(k)
    vtr = t_bf(v)
    imt = t_bf(img_abs)

    img = np.asarray(img_abs, f32)
    augf = np.empty((B, L, H * AUGW), f32)
    augf.reshape(B, L, H, AUGW)[..., :64] = img.reshape(B, L, H, 64)
    augf.reshape(B, L, H, AUGW)[..., 64] = 1.0
    augv = augf.astype(bf16)

    def keepT(m):  # [B, 1, L, L] bool -> (1-m)^T bf16
        kf = 1.0 - np.asarray(m, f32)[:, 0]
        return np.ascontiguousarray(np.swapaxes(kf, 1, 2)).astype(bf16)

    kp1 = keepT(abs_mask)
    kp2 = keepT(mask)

    def wT(w):
        return np.ascontiguousarray(np.asarray(w, f32).T).astype(bf16)

    wqs, wks, wvs, wms = wT(Wq), wT(Wk), wT(Wv), wT(Wm)
    ident = np.eye(128, dtype=bf16)
    idshift = np.zeros((64, 128), dtype=bf16)
    idshift[np.arange(64), 64 + np.arange(64)] = 1

    in_maps = []
    for c in range(NCORES):
        s = slice(c * BPC, (c + 1) * BPC)
        in_maps.append(
            {
                "qt": qt[s],
                "kt": ktr[s],
                "vt": vtr[s],
                "imt": imt[s],
                "aug": augv[s],
                "kp1": kp1[s],
                "kp2": kp2[s],
                "wq": wqs,
                "wk": wks,
                "wv": wvs,
                "wm": wms,
                "idt": ident,
                "ids": idshift,
            }
        )
    return in_maps


def kernel(v, k, q, img_abs, Wv, Wk, Wq, Wm, abs_mask, mask, _trace=False):
    _ensure_concourse()
    from concourse.bass_utils import run_bass_kernel_spmd

    in_maps = _prep_inputs(v, k, q, img_abs, Wv, Wk, Wq, Wm, abs_mask, mask)
    nc = _get_nc()
    res = run_bass_kernel_spmd(nc, in_maps, core_ids=list(range(NCORES)), trace=_trace)
    outp = np.concatenate([res.results[i]["out"] for i in range(NCORES)], axis=0)
    outp = np.asarray(outp, np.float32)
    if _trace:
        _CACHE["last_result"] = res
    return outp
